# revision 61
# baseline (speedup 1.0000x reference)
"""Cross-attention kernel for Trainium2 (8 NeuronCores, batch-parallel).

Reference computation (per batch element b):
    q = x @ Wq + bq            # [T, E]
    k = y @ Wk + bk            # [S, E]
    v = y @ Wv + bv            # [S, E]
    per head h (D=80): scores = q_h @ k_h.T / sqrt(D); A = softmax(scores)
    attn = concat_h(A @ v_h)   # [T, E]
    out = attn @ Wo + bo       # [T, E]

Sharding: batch (8) across the 8 cores, one batch element per core.

Design (vs the f32 baseline; cost-model time 210us -> 134us):
  - bf16 for every DMA'd tensor (x, y, weights, output) and the matmul
    streams; PSUM accumulation stays f32, softmax sums stay f32. Halves
    HBM traffic and SBUF pressure (rel tolerance is 2e-2, measured HW
    error ~4.4e-3).
  - softmax denominators via gpsimd partition_all_reduce +
    reciprocal_approx_fast + normalize multiplies alternating DVE/Pool:
    removes the baseline's 8 per-chunk sum-of-exp matmuls and its
    latency-heavy DMA+partition_broadcast chain.
  - heads whose 80 features straddle a 128-partition tile boundary
    (h1/h3/h4/h6) get their k and q repacked into dedicated 128-row
    tiles (SBUF-SBUF DMA, one chunk ahead), collapsing their two score
    fragments into ONE matmul: scores cost 8 MMs/chunk instead of 12.
  - one big DMA per chunk for x loads and output stores ([128, 5, tw]
    rearranged views) instead of 5 small ones (HWDGE fixed cost paid
    once); weight loads split by column block so Vproj/Kproj/Qproj
    start as soon as their slice lands; wv/wk/wo loads and the output
    stores issue from the ACT-engine HWDGE ring (qActDynamicHW) so the
    two physical descriptor-generation rings run in parallel.
  - software pipelining: per chunk the PE stream is scores(i) ->
    Oproj(i-1) -> Qproj(i+1) -> AV(i), so the softmax normalization
    chain of chunk i hides under ~10us of projection matmuls.
  - PE warmup matmuls during the initial weight-DMA fill so the
    p-state/HAM ramp completes before real work arrives; narrow first/
    last chunks shrink pipeline fill and drain; the last chunk
    interleaves its O-projection j-groups with the AV tiles.

On-chip layout is feature-major (x and the output are transposed on the
host): xt = x[b].T [E, T]; q' = Wq.T@X' [E, T]; per-head zero-padded
k/v staging tiles (80-dim heads vs 128-partition tiles).
"""

import numpy as np

import concourse.bass as bass
import concourse.bass_isa as bass_isa
import concourse.mybir as mybir
import concourse.tile as tile
from concourse import bacc
from concourse.bass_utils import run_bass_kernel_spmd

F32 = mybir.dt.float32
F32R = mybir.dt.float32r
BF16 = mybir.dt.bfloat16
AF = mybir.ActivationFunctionType

B, T, E, CR, H, D, S = 8, 4096, 640, 768, 8, 80, 77
TC = 512
# narrow edge chunks: chunk 0 starts sooner (less DMA before the pipeline
# start) and the last chunk's O-projection + store can't overlap anything.
CHUNKS = ([(0, 256)] + [(256 + 512 * i, 512) for i in range(7)]
          + [(3840, 256)])
NT = len(CHUNKS)
EJ = E // 128            # 5 e-tiles
CJ = CR // 128           # 6 cross-dim tiles
SCALE = float(1.0 / np.sqrt(D))
NWARM = 80               # PE warmup matmuls during DMA fill
AT_ACT_N = 4             # at-copies j < AT_ACT_N go to ACT, rest to DVE


def _frags():
    fr = []
    for h in range(H):
        e0, e1 = D * h, D * h + D
        for j in range(e0 // 128, (e1 - 1) // 128 + 1):
            p0, p1 = max(0, e0 - 128 * j), min(128, e1 - 128 * j)
            fr.append((h, j, p0, p1))
    return fr


FRAGS = _frags()         # 12 fragments
NF = len(FRAGS)
# heads whose 80 features straddle a 128-partition tile boundary (two
# fragments); their scores run as ONE matmul against a repacked q tile
STRADDLE = []            # (h, fiA, fiB, j, p0, p1): fragA=(j,p0,128), fragB=(j+1,0,p1)
for h in range(H):
    frs = [(fi, f) for fi, f in enumerate(FRAGS) if f[0] == h]
    if len(frs) == 2:
        (fiA, (_, jA, p0A, _)), (fiB, (_, jB, _, p1B)) = frs
        STRADDLE.append((h, fiA, fiB, jA, p0A, p1B))
SINGLE = [(h, [fi for fi, f in enumerate(FRAGS) if f[0] == h][0])
          for h in range(H)
          if len([fi for fi, f in enumerate(FRAGS) if f[0] == h]) == 1]


def _emit(nc, tc, dr):
    import contextlib

    ctx = contextlib.ExitStack()
    with ctx:
        cpool = ctx.enter_context(tc.tile_pool(name="const", bufs=1))
        pq = ctx.enter_context(tc.tile_pool(name="pq", bufs=2, space="PSUM"))
        psc = ctx.enter_context(tc.tile_pool(name="psc", bufs=2, space="PSUM"))
        pav = ctx.enter_context(tc.tile_pool(name="pav", bufs=2, space="PSUM"))
        pop = ctx.enter_context(tc.tile_pool(name="pop", bufs=2, space="PSUM"))
        xpool = ctx.enter_context(tc.tile_pool(name="xpool", bufs=2))
        qpool = ctx.enter_context(tc.tile_pool(name="qpool", bufs=2))
        apool = ctx.enter_context(tc.tile_pool(name="apool", bufs=2))
        arpool = ctx.enter_context(tc.tile_pool(name="arpool", bufs=3))
        atpool = ctx.enter_context(tc.tile_pool(name="atpool", bufs=2))
        opool = ctx.enter_context(tc.tile_pool(name="opool", bufs=2))

        xtr = dr["xt"].rearrange("(b p) c -> p b c", p=128)
        otr = dr["ot"].rearrange("(b p) c -> p b c", p=128)

        # ---- PE warmup: keep the array busy through the DMA fill ----
        wtile = cpool.tile([128, 128], BF16, tag="warm", name="warm")
        nc.vector.memset(wtile[:], 0.0)
        wps = pop.tile([128, 128], F32, tag="op", name="warmps")
        for _ in range(NWARM):
            nc.tensor.matmul(wps[:], wtile[:], wtile[:], start=True, stop=True)

        # ---- weight/const loads (order = DMA queue order). The big
        # weights are split by column block so dependent PE groups can
        # start as soon as their slice lands.
        def loadw(name, src2, nblk, cols, c0=0, c1=None, dt=BF16,
                  eng=None):
            c1 = cols if c1 is None else c1
            t = cpool.tile([128, nblk, c1 - c0], dt, tag=name, name=name)
            (eng or nc.sync).dma_start(
                t[:], src2.rearrange("(b p) c -> p b c", p=128)[:, :, c0:c1])
            return t

        ytp_t = loadw("yt", dr["yt"], CJ, S)
        consts2 = cpool.tile([1, E + S], BF16, tag="consts2", name="consts2")
        nc.sync.dma_start(consts2[:], dr["consts2"])
        consts = cpool.tile([128, 27], F32, tag="consts", name="consts")
        nc.sync.dma_start(consts[:], dr["consts"])
        # wv via the ACT-engine HWDGE ring (qActDynamicHW on HW): its
        # descriptor generation runs parallel to the SP-ring loads
        wv_a = loadw("wva", dr["wv"], CJ, E, 0, 512, eng=nc.scalar)
        wv_b = loadw("wvb", dr["wv"], CJ, E, 512, E, eng=nc.scalar)
        wk_a = loadw("wka", dr["wk"], CJ, E, 0, 256, eng=nc.scalar)
        wk_b = loadw("wkb", dr["wk"], CJ, E, 256, E, eng=nc.scalar)
        wq_a = loadw("wqa", dr["wq"], EJ, E, 0, 256)
        xp0 = xpool.tile([128, EJ, TC], BF16, tag="xp", name="xp0")
        nc.sync.dma_start(xp0[0:128, 0:EJ, 0:CHUNKS[0][1]],
                          xtr[:, :, CHUNKS[0][0]:CHUNKS[0][0] + CHUNKS[0][1]])
        wq_b = loadw("wqb", dr["wq"], EJ, E, 256, E)
        xp1 = xpool.tile([128, EJ, TC], BF16, tag="xp", name="xp1")
        nc.sync.dma_start(xp1[0:128, 0:EJ, 0:CHUNKS[1][1]],
                          xtr[:, :, CHUNKS[1][0]:CHUNKS[1][0] + CHUNKS[1][1]])
        # qpk(0) DMAs, wo + remaining x chunks are issued further down

        def wq_col(j, c):
            return (wq_a[:, c, 128 * j:128 * (j + 1)] if j < 2
                    else wq_b[:, c, 128 * (j - 2):128 * (j - 1)])

        def wk_col(j, c):
            return (wk_a[:, c, 128 * j:128 * (j + 1)] if j < 2
                    else wk_b[:, c, 128 * (j - 2):128 * (j - 1)])

        bqt = consts[:, 0:EJ]
        bkt = consts[:, EJ:2 * EJ]
        bot = consts[:, 2 * EJ:3 * EJ]
        kmask = consts[:, 3 * EJ:3 * EJ + NF]
        bvr = consts2[:, 0:E]
        ones77 = consts2[:, E:E + S]

        # ---- V projection -> vb fragments (zero-padded) ----
        # vb holds, per fragment, a [S, 128] slab that is zero outside the
        # head's partition range; built by memset + free-dim-offset copies.
        vb = cpool.tile([S, NF * 128], BF16, tag="vb", name="vb")
        nc.gpsimd.memset(vb[:], 0.0)

        def emit_vproj():
            for (n0, n1), wv_t in (((0, 512), wv_a), ((512, E), wv_b)):
                vp = psc.tile([S, n1 - n0], F32, tag="sc")
                for c in range(CJ):
                    nc.tensor.matmul(vp[:], ytp_t[:, c, :],
                                     wv_t[:, c, :],
                                     start=(c == 0), stop=False)
                nc.tensor.matmul(vp[:], ones77[:], bvr[:, n0:n1],
                                 start=False, stop=True)
                for fi, (h, j, p0, p1) in enumerate(FRAGS):
                    c0 = 128 * j
                    if not (n0 <= c0 and c0 + 128 <= n1):
                        continue
                    # on ACT (idle during setup): keeps DVE clear for the
                    # kstage ops that gate Kproj's psum recycling
                    nc.scalar.activation(
                        vb[:, 128 * fi + p0:128 * fi + p1],
                        vp[:, c0 - n0 + p0:c0 - n0 + p1], AF.Copy)

        # ---- K projection -> zero-masked per-fragment staging tiles ----
        kstage = [cpool.tile([128, S], BF16, tag=f"ks{fi}", name=f"ks{fi}")
                  for fi in range(NF)]

        def emit_kproj():
            for j in range(EJ):
                # alternate psum pools (pav is idle during setup) so the
                # kstage DVE ops don't gate the bank recycling
                kpool = pq if j % 2 == 0 else pav
                kp = kpool.tile([128, S], F32, tag="qp" if j % 2 == 0
                                else "av")
                for c in range(CJ):
                    nc.tensor.matmul(kp[:], wk_col(j, c),
                                     ytp_t[:, c, :], start=(c == 0),
                                     stop=(c == CJ - 1))
                for fi, (h, jj, p0, p1) in enumerate(FRAGS):
                    if jj != j:
                        continue
                    nc.vector.tensor_scalar(kstage[fi][:], kp[:],
                                            bkt[:, j:j + 1],
                                            kmask[:, fi:fi + 1],
                                            mybir.AluOpType.add,
                                            mybir.AluOpType.mult)

        def emit_qproj(xp, tw, qpools=None):
            qs = []
            for j in range(EJ):
                pool, ptag = (qpools[j] if qpools
                              else (pq, "qp"))
                qp = pool.tile([128, TC], F32, tag=ptag, name=f"qp{j}")
                for c in range(EJ):
                    nc.tensor.matmul(qp[0:128, 0:tw],
                                     wq_col(j, c),
                                     xp[0:128, c, 0:tw],
                                     start=(c == 0), stop=(c == EJ - 1))
                q = qpool.tile([128, TC], BF16, tag=f"q{j}", name=f"q{j}")
                nc.vector.tensor_scalar_add(q[0:128, 0:tw], qp[0:128, 0:tw],
                                            bqt[:, j:j + 1])
                qs.append(q)
            return qs

        # ---- packed k/q staging for straddling heads: their two score
        # fragments collapse to ONE matmul against a q tile whose rows
        # 0:80 are the head's features (repacked via SBUF-SBUF DMA).
        kpk = {}
        qpk = {}
        for (h, fiA, fiB, j, p0, p1) in STRADDLE:
            kpk[h] = cpool.tile([128, S], BF16, tag=f"kpk{h}", name=f"kpk{h}")
            qpk[h] = cpool.tile([128, TC], BF16, tag=f"qpk{h}",
                                name=f"qpk{h}")
            nc.gpsimd.memset(kpk[h][:], 0.0)
            nc.gpsimd.memset(qpk[h][:], 0.0)

        def emit_kpk():
            for (h, fiA, fiB, j, p0, p1) in STRADDLE:
                nc.sync.dma_start(kpk[h][0:128 - p0, :],
                                  kstage[fiA][p0:128, :])
                nc.sync.dma_start(kpk[h][128 - p0:D, :],
                                  kstage[fiB][0:p1, :])

        def emit_qpk(qs, tw):
            for (h, fiA, fiB, j, p0, p1) in STRADDLE:
                nc.sync.dma_start(qpk[h][0:128 - p0, 0:tw],
                                  qs[j][p0:128, 0:tw])
                nc.sync.dma_start(qpk[h][128 - p0:D, 0:tw],
                                  qs[j + 1][0:p1, 0:tw])

        def _softmax_tail(h, sc, tw, aps):
            """exp -> allreduce(sumexp) -> approx-recip -> normalize.
            The normalize multiplies alternate DVE/Pool to spread the
            elementwise load (DVE is near-saturated in steady state)."""
            a = apool.tile([S, TC], BF16, tag=f"a{h}", name=f"a{h}")
            nc.scalar.activation(a[0:S, 0:tw], sc[0:S, 0:tw], AF.Exp,
                                 scale=SCALE)
            ar = arpool.tile([S, TC], F32, tag="ar")
            nc.gpsimd.partition_all_reduce(
                ar[0:S, 0:tw], a[0:S, 0:tw], channels=S,
                reduce_op=bass_isa.ReduceOp.add)
            nc.vector.reciprocal_approx_fast(ar[0:S, 0:tw], ar[0:S, 0:tw])
            eng = nc.vector if h % 2 == 0 else nc.gpsimd
            eng.tensor_mul(a[0:S, 0:tw], a[0:S, 0:tw], ar[0:S, 0:tw])
            aps[h] = a

        def emit_scores_all(qs, tw, aps):
            """fragment-accumulation scores for all heads (chunk 0 only,
            before the packed q staging pipeline is primed)."""
            for h in range(H):
                frs = [(fi, f) for fi, f in enumerate(FRAGS) if f[0] == h]
                sc = psc.tile([S, TC], F32, tag="sc")
                for i, (fi, (hh, j, p0, p1)) in enumerate(frs):
                    nc.tensor.matmul(sc[0:S, 0:tw], kstage[fi][:],
                                     qs[j][0:128, 0:tw],
                                     start=(i == 0), stop=(i == len(frs) - 1))
                _softmax_tail(h, sc, tw, aps)

        def emit_scores_singles(qs, tw, aps):
            for (h, fi) in SINGLE:
                j = FRAGS[fi][1]
                sc = psc.tile([S, TC], F32, tag="sc")
                nc.tensor.matmul(sc[0:S, 0:tw], kstage[fi][:],
                                 qs[j][0:128, 0:tw], start=True, stop=True)
                _softmax_tail(h, sc, tw, aps)

        def emit_scores_packed(tw, aps):
            for (h, fiA, fiB, j, p0, p1) in STRADDLE:
                sc = psc.tile([S, TC], F32, tag="sc")
                nc.tensor.matmul(sc[0:S, 0:tw], kpk[h][:],
                                 qpk[h][0:128, 0:tw], start=True, stop=True)
                _softmax_tail(h, sc, tw, aps)

        def emit_av_tile(j, aps, tw, dve_at=False):
            av = pav.tile([128, TC], F32, tag="av")
            frs = [(fi, f) for fi, f in enumerate(FRAGS) if f[1] == j]
            for i, (fi, (h, jj, p0, p1)) in enumerate(frs):
                nc.tensor.matmul(av[0:128, 0:tw],
                                 vb[:, 128 * fi:128 * (fi + 1)],
                                 aps[h][0:S, 0:tw],
                                 start=(i == 0), stop=(i == len(frs) - 1))
            at = atpool.tile([128, TC], BF16, tag=f"at{j}", name=f"at{j}")
            if j < AT_ACT_N and not dve_at:
                nc.scalar.activation(at[0:128, 0:tw], av[0:128, 0:tw],
                                     AF.Copy)
            else:
                nc.vector.tensor_copy(at[0:128, 0:tw], av[0:128, 0:tw])
            return at

        def emit_oproj(attn, t0, tw, store_eng=None):
            obt = opool.tile([128, EJ, TC], BF16, tag="ob")
            for p in range(EJ):
                op = pop.tile([128, TC], F32, tag="op")
                for j in range(EJ):
                    nc.tensor.matmul(op[0:128, 0:tw],
                                     wo_t[:, j, 128 * p:128 * (p + 1)],
                                     attn[j][0:128, 0:tw],
                                     start=(j == 0), stop=(j == EJ - 1))
                nc.scalar.activation(obt[0:128, p, 0:tw], op[0:128, 0:tw],
                                     AF.Identity, bias=bot[:, p:p + 1])
            # store on the ACT HWDGE ring: frees the SP ring for x/qpk
            # prefetches and pairs naturally with the ob copies above
            (store_eng or nc.scalar).dma_start(otr[:, :, t0:t0 + tw],
                                               obt[0:128, 0:EJ, 0:tw])

        # ---- setup PE work: Vproj (needs yt+wv), Kproj (wk), Q(0) ----
        emit_vproj()
        emit_kproj()
        emit_kpk()
        qs = emit_qproj(xp0, CHUNKS[0][1])
        wo_t = loadw("wo", dr["wo"], EJ, E, eng=nc.scalar)

        attn_prev = None
        tprev = None
        xps = [xp0, xp1]
        for it in range(NT):
            t0, tw = CHUNKS[it]
            # prefetch x(i+2) (x0/x1 loaded during setup)
            if it + 2 < NT:
                nt0, ntw = CHUNKS[it + 2]
                xp_next = xpool.tile([128, EJ, TC], BF16, tag="xp")
                nc.sync.dma_start(xp_next[0:128, 0:EJ, 0:ntw],
                                  xtr[:, :, nt0:nt0 + ntw])
                xps.append(xp_next)

            aps = {}
            if it == 0:
                emit_scores_all(qs, tw, aps)
            else:
                emit_scores_singles(qs, tw, aps)
                emit_scores_packed(tw, aps)

            if it > 0:
                # near the narrow tail chunks the ACT SEQ is congested;
                # route those stores back to the SP ring
                emit_oproj(attn_prev, tprev[0], tprev[1],
                           store_eng=nc.sync if it >= NT - 2 else None)

            if it < NT - 1:
                # Q(i+1) before AV(i): together with Oproj(i-1) it covers
                # the softmax normalization chain latency of chunk i.
                qs = emit_qproj(xps[it + 1], CHUNKS[it + 1][1])
                emit_qpk(qs, CHUNKS[it + 1][1])
                attn = [emit_av_tile(j, aps, tw) for j in range(EJ)]
            else:
                # last chunk: j-outer Oproj interleaved with AV tiles,
                # staggered by one so the at-copy latency is hidden.
                ops = [pop.tile([128, TC], F32, tag="op", name=f"opl{p}")
                       for p in range(2)]
                ops += [pq.tile([128, TC], F32, tag="qp", name=f"opl{p + 2}")
                        for p in range(2)]
                ops += [psc.tile([128, TC], F32, tag="sc", name="opl4")]
                attn = []
                obt = opool.tile([128, EJ, TC], BF16, tag="ob")

                def last_o_group(j):
                    for p in range(EJ):
                        nc.tensor.matmul(ops[p][0:128, 0:tw],
                                         wo_t[:, j, 128 * p:128 * (p + 1)],
                                         attn[j][0:128, 0:tw],
                                         start=(j == 0), stop=(j == EJ - 1))

                for j in range(EJ):
                    # at-copies on DVE: it is idle in the last chunk (no
                    # Q(i+1) bias work) while ACT still drains exp's
                    attn.append(emit_av_tile(j, aps, tw, dve_at=True))
                    if j >= 1:
                        last_o_group(j - 1)
                # final accumulation group: emit each p's closing matmul,
                # its psum->sbuf bias copy, and the store as soon as ready
                for p in range(EJ):
                    nc.tensor.matmul(ops[p][0:128, 0:tw],
                                     wo_t[:, EJ - 1, 128 * p:128 * (p + 1)],
                                     attn[EJ - 1][0:128, 0:tw],
                                     start=False, stop=True)
                    if p % 2 == 0:
                        nc.scalar.activation(obt[0:128, p, 0:tw],
                                             ops[p][0:128, 0:tw],
                                             AF.Identity,
                                             bias=bot[:, p:p + 1])
                    else:
                        nc.vector.tensor_scalar_add(obt[0:128, p, 0:tw],
                                                    ops[p][0:128, 0:tw],
                                                    bot[:, p:p + 1])
                    if p == 2:
                        nc.scalar.dma_start(otr[:, 0:3, t0:t0 + tw],
                                            obt[0:128, 0:3, 0:tw])
                nc.sync.dma_start(otr[:, 3:EJ, t0:t0 + tw],
                                  obt[0:128, 3:EJ, 0:tw])
            attn_prev, tprev = attn, (t0, tw)


def build_program():
    nc = bacc.Bacc("TRN2", target_bir_lowering=False, debug=False,
                   num_devices=B)
    dr = {}

    def din(name, shape, dt):
        dr[name] = nc.dram_tensor(name, shape, dt, kind="ExternalInput")
        return dr[name]

    din("xt", [E, T], BF16)
    din("yt", [CR, S], BF16)
    din("wq", [E, E], BF16)
    din("wk", [CR, E], BF16)
    din("wv", [CR, E], BF16)
    din("wo", [E, E], BF16)
    din("consts", [128, 27], F32)
    din("consts2", [1, E + S], BF16)
    dr["ot"] = nc.dram_tensor("ot", [E, T], BF16, kind="ExternalOutput")

    with tile.TileContext(nc) as tc:
        _emit(nc, tc, {k: v[:] for k, v in dr.items()})
    nc.compile()
    return nc


def make_in_maps(x, y, Wq, bq, Wk, bk, Wv, bv, Wo, bo):
    import ml_dtypes
    BF = ml_dtypes.bfloat16

    def fb(a):
        return np.ascontiguousarray(np.asarray(a, np.float32).astype(BF))

    consts = np.zeros((128, 27), np.float32)
    consts[:, 0:EJ] = np.asarray(bq, np.float32).reshape(EJ, 128).T
    consts[:, EJ:2 * EJ] = np.asarray(bk, np.float32).reshape(EJ, 128).T
    consts[:, 2 * EJ:3 * EJ] = np.asarray(bo, np.float32).reshape(EJ, 128).T
    for fi, (h, j, p0, p1) in enumerate(FRAGS):
        consts[p0:p1, 3 * EJ + fi] = 1.0
    consts2 = np.zeros((1, E + S), np.float32)
    consts2[0, 0:E] = np.asarray(bv, np.float32)
    consts2[0, E:E + S] = 1.0

    shared = dict(
        wq=fb(Wq), wk=fb(Wk), wv=fb(Wv), wo=fb(Wo),
        consts=consts, consts2=fb(consts2),
    )
    x = np.asarray(x, np.float32)
    y = np.asarray(y, np.float32)
    in_maps = []
    for b in range(B):
        m = dict(shared)
        m["xt"] = fb(x[b].T)
        m["yt"] = fb(y[b].T)
        in_maps.append(m)
    return in_maps


def assemble_output(results):
    return np.stack(
        [np.asarray(results[b]["ot"]).astype(np.float32).T
         for b in range(B)], axis=0)


_PROG = None


def _prog():
    global _PROG
    if _PROG is None:
        _PROG = build_program()
    return _PROG


def kernel(x, y, Wq, bq, Wk, bk, Wv, bv, Wo, bo):
    nc = _prog()
    in_maps = make_in_maps(x, y, Wq, bq, Wk, bk, Wv, bv, Wo, bo)
    res = run_bass_kernel_spmd(nc, in_maps, core_ids=list(range(B)))
    return assemble_output(res.results)


# revision 62
# speedup vs baseline: 1.0012x; 1.0012x over previous
"""Cross-attention kernel for Trainium2 (8 NeuronCores, batch-parallel).

Reference computation (per batch element b):
    q = x @ Wq + bq            # [T, E]
    k = y @ Wk + bk            # [S, E]
    v = y @ Wv + bv            # [S, E]
    per head h (D=80): scores = q_h @ k_h.T / sqrt(D); A = softmax(scores)
    attn = concat_h(A @ v_h)   # [T, E]
    out = attn @ Wo + bo       # [T, E]

Sharding: batch (8) across the 8 cores, one batch element per core.

Design (vs the f32 baseline; cost-model time 210us -> 134us):
  - bf16 for every DMA'd tensor (x, y, weights, output) and the matmul
    streams; PSUM accumulation stays f32, softmax sums stay f32. Halves
    HBM traffic and SBUF pressure (rel tolerance is 2e-2, measured HW
    error ~4.4e-3).
  - softmax denominators via gpsimd partition_all_reduce +
    reciprocal_approx_fast + normalize multiplies alternating DVE/Pool:
    removes the baseline's 8 per-chunk sum-of-exp matmuls and its
    latency-heavy DMA+partition_broadcast chain.
  - heads whose 80 features straddle a 128-partition tile boundary
    (h1/h3/h4/h6) get their k and q repacked into dedicated 128-row
    tiles (SBUF-SBUF DMA, one chunk ahead), collapsing their two score
    fragments into ONE matmul: scores cost 8 MMs/chunk instead of 12.
  - one big DMA per chunk for x loads and output stores ([128, 5, tw]
    rearranged views) instead of 5 small ones (HWDGE fixed cost paid
    once); weight loads split by column block so Vproj/Kproj/Qproj
    start as soon as their slice lands; wv/wk/wo loads and the output
    stores issue from the ACT-engine HWDGE ring (qActDynamicHW) so the
    two physical descriptor-generation rings run in parallel.
  - software pipelining: per chunk the PE stream is scores(i) ->
    Oproj(i-1) -> Qproj(i+1) -> AV(i), so the softmax normalization
    chain of chunk i hides under ~10us of projection matmuls.
  - PE warmup matmuls during the initial weight-DMA fill so the
    p-state/HAM ramp completes before real work arrives; narrow first/
    last chunks shrink pipeline fill and drain; the last chunk
    interleaves its O-projection j-groups with the AV tiles.

On-chip layout is feature-major (x and the output are transposed on the
host): xt = x[b].T [E, T]; q' = Wq.T@X' [E, T]; per-head zero-padded
k/v staging tiles (80-dim heads vs 128-partition tiles).
"""

import numpy as np

import concourse.bass as bass
import concourse.bass_isa as bass_isa
import concourse.mybir as mybir
import concourse.tile as tile
from concourse import bacc
from concourse.bass_utils import run_bass_kernel_spmd

F32 = mybir.dt.float32
F32R = mybir.dt.float32r
BF16 = mybir.dt.bfloat16
AF = mybir.ActivationFunctionType

B, T, E, CR, H, D, S = 8, 4096, 640, 768, 8, 80, 77
TC = 512
# narrow edge chunks: chunk 0 starts sooner (less DMA before the pipeline
# start) and the last chunk's O-projection + store can't overlap anything.
CHUNKS = ([(0, 256)] + [(256 + 512 * i, 512) for i in range(7)]
          + [(3840, 256)])
NT = len(CHUNKS)
EJ = E // 128            # 5 e-tiles
CJ = CR // 128           # 6 cross-dim tiles
SCALE = float(1.0 / np.sqrt(D))
NWARM = 77               # PE warmup matmuls during DMA fill (75 is the
                         # sim optimum but sits one MM from a 1.3us
                         # phase-alignment cliff; 77 keeps margin)
AT_ACT_N = 4             # at-copies j < AT_ACT_N go to ACT, rest to DVE


def _frags():
    fr = []
    for h in range(H):
        e0, e1 = D * h, D * h + D
        for j in range(e0 // 128, (e1 - 1) // 128 + 1):
            p0, p1 = max(0, e0 - 128 * j), min(128, e1 - 128 * j)
            fr.append((h, j, p0, p1))
    return fr


FRAGS = _frags()         # 12 fragments
NF = len(FRAGS)
# heads whose 80 features straddle a 128-partition tile boundary (two
# fragments); their scores run as ONE matmul against a repacked q tile
STRADDLE = []            # (h, fiA, fiB, j, p0, p1): fragA=(j,p0,128), fragB=(j+1,0,p1)
for h in range(H):
    frs = [(fi, f) for fi, f in enumerate(FRAGS) if f[0] == h]
    if len(frs) == 2:
        (fiA, (_, jA, p0A, _)), (fiB, (_, jB, _, p1B)) = frs
        STRADDLE.append((h, fiA, fiB, jA, p0A, p1B))
SINGLE = [(h, [fi for fi, f in enumerate(FRAGS) if f[0] == h][0])
          for h in range(H)
          if len([fi for fi, f in enumerate(FRAGS) if f[0] == h]) == 1]


def _emit(nc, tc, dr):
    import contextlib

    ctx = contextlib.ExitStack()
    with ctx:
        cpool = ctx.enter_context(tc.tile_pool(name="const", bufs=1))
        pq = ctx.enter_context(tc.tile_pool(name="pq", bufs=2, space="PSUM"))
        psc = ctx.enter_context(tc.tile_pool(name="psc", bufs=2, space="PSUM"))
        pav = ctx.enter_context(tc.tile_pool(name="pav", bufs=2, space="PSUM"))
        pop = ctx.enter_context(tc.tile_pool(name="pop", bufs=2, space="PSUM"))
        xpool = ctx.enter_context(tc.tile_pool(name="xpool", bufs=2))
        qpool = ctx.enter_context(tc.tile_pool(name="qpool", bufs=2))
        apool = ctx.enter_context(tc.tile_pool(name="apool", bufs=2))
        arpool = ctx.enter_context(tc.tile_pool(name="arpool", bufs=3))
        atpool = ctx.enter_context(tc.tile_pool(name="atpool", bufs=2))
        opool = ctx.enter_context(tc.tile_pool(name="opool", bufs=2))

        xtr = dr["xt"].rearrange("(b p) c -> p b c", p=128)
        otr = dr["ot"].rearrange("(b p) c -> p b c", p=128)

        # ---- PE warmup: keep the array busy through the DMA fill ----
        wtile = cpool.tile([128, 128], BF16, tag="warm", name="warm")
        nc.vector.memset(wtile[:], 0.0)
        wps = pop.tile([128, 128], F32, tag="op", name="warmps")
        for _ in range(NWARM):
            nc.tensor.matmul(wps[:], wtile[:], wtile[:], start=True, stop=True)

        # ---- weight/const loads (order = DMA queue order). The big
        # weights are split by column block so dependent PE groups can
        # start as soon as their slice lands.
        def loadw(name, src2, nblk, cols, c0=0, c1=None, dt=BF16,
                  eng=None):
            c1 = cols if c1 is None else c1
            t = cpool.tile([128, nblk, c1 - c0], dt, tag=name, name=name)
            (eng or nc.sync).dma_start(
                t[:], src2.rearrange("(b p) c -> p b c", p=128)[:, :, c0:c1])
            return t

        ytp_t = loadw("yt", dr["yt"], CJ, S)
        consts2 = cpool.tile([1, E + S], BF16, tag="consts2", name="consts2")
        nc.sync.dma_start(consts2[:], dr["consts2"])
        consts = cpool.tile([128, 27], F32, tag="consts", name="consts")
        nc.sync.dma_start(consts[:], dr["consts"])
        # wv via the ACT-engine HWDGE ring (qActDynamicHW on HW): its
        # descriptor generation runs parallel to the SP-ring loads
        wv_a = loadw("wva", dr["wv"], CJ, E, 0, 512, eng=nc.scalar)
        wv_b = loadw("wvb", dr["wv"], CJ, E, 512, E, eng=nc.scalar)
        wk_a = loadw("wka", dr["wk"], CJ, E, 0, 256, eng=nc.scalar)
        wk_b = loadw("wkb", dr["wk"], CJ, E, 256, E, eng=nc.scalar)
        wq_a = loadw("wqa", dr["wq"], EJ, E, 0, 256)
        xp0 = xpool.tile([128, EJ, TC], BF16, tag="xp", name="xp0")
        nc.sync.dma_start(xp0[0:128, 0:EJ, 0:CHUNKS[0][1]],
                          xtr[:, :, CHUNKS[0][0]:CHUNKS[0][0] + CHUNKS[0][1]])
        wq_b = loadw("wqb", dr["wq"], EJ, E, 256, E)
        xp1 = xpool.tile([128, EJ, TC], BF16, tag="xp", name="xp1")
        nc.sync.dma_start(xp1[0:128, 0:EJ, 0:CHUNKS[1][1]],
                          xtr[:, :, CHUNKS[1][0]:CHUNKS[1][0] + CHUNKS[1][1]])
        # qpk(0) DMAs, wo + remaining x chunks are issued further down

        def wq_col(j, c):
            return (wq_a[:, c, 128 * j:128 * (j + 1)] if j < 2
                    else wq_b[:, c, 128 * (j - 2):128 * (j - 1)])

        def wk_col(j, c):
            return (wk_a[:, c, 128 * j:128 * (j + 1)] if j < 2
                    else wk_b[:, c, 128 * (j - 2):128 * (j - 1)])

        bqt = consts[:, 0:EJ]
        bkt = consts[:, EJ:2 * EJ]
        bot = consts[:, 2 * EJ:3 * EJ]
        kmask = consts[:, 3 * EJ:3 * EJ + NF]
        bvr = consts2[:, 0:E]
        ones77 = consts2[:, E:E + S]

        # ---- V projection -> vb fragments (zero-padded) ----
        # vb holds, per fragment, a [S, 128] slab that is zero outside the
        # head's partition range; built by memset + free-dim-offset copies.
        vb = cpool.tile([S, NF * 128], BF16, tag="vb", name="vb")
        nc.gpsimd.memset(vb[:], 0.0)

        def emit_vproj():
            for (n0, n1), wv_t in (((0, 512), wv_a), ((512, E), wv_b)):
                vp = psc.tile([S, n1 - n0], F32, tag="sc")
                for c in range(CJ):
                    nc.tensor.matmul(vp[:], ytp_t[:, c, :],
                                     wv_t[:, c, :],
                                     start=(c == 0), stop=False)
                nc.tensor.matmul(vp[:], ones77[:], bvr[:, n0:n1],
                                 start=False, stop=True)
                for fi, (h, j, p0, p1) in enumerate(FRAGS):
                    c0 = 128 * j
                    if not (n0 <= c0 and c0 + 128 <= n1):
                        continue
                    # on ACT (idle during setup): keeps DVE clear for the
                    # kstage ops that gate Kproj's psum recycling
                    nc.scalar.activation(
                        vb[:, 128 * fi + p0:128 * fi + p1],
                        vp[:, c0 - n0 + p0:c0 - n0 + p1], AF.Copy)

        # ---- K projection -> zero-masked per-fragment staging tiles ----
        kstage = [cpool.tile([128, S], BF16, tag=f"ks{fi}", name=f"ks{fi}")
                  for fi in range(NF)]

        def emit_kproj():
            for j in range(EJ):
                # alternate psum pools (pav is idle during setup) so the
                # kstage DVE ops don't gate the bank recycling
                kpool = pq if j % 2 == 0 else pav
                kp = kpool.tile([128, S], F32, tag="qp" if j % 2 == 0
                                else "av")
                for c in range(CJ):
                    nc.tensor.matmul(kp[:], wk_col(j, c),
                                     ytp_t[:, c, :], start=(c == 0),
                                     stop=(c == CJ - 1))
                for fi, (h, jj, p0, p1) in enumerate(FRAGS):
                    if jj != j:
                        continue
                    nc.vector.tensor_scalar(kstage[fi][:], kp[:],
                                            bkt[:, j:j + 1],
                                            kmask[:, fi:fi + 1],
                                            mybir.AluOpType.add,
                                            mybir.AluOpType.mult)

        def emit_qproj(xp, tw, qpools=None):
            qs = []
            for j in range(EJ):
                pool, ptag = (qpools[j] if qpools
                              else (pq, "qp"))
                qp = pool.tile([128, TC], F32, tag=ptag, name=f"qp{j}")
                for c in range(EJ):
                    nc.tensor.matmul(qp[0:128, 0:tw],
                                     wq_col(j, c),
                                     xp[0:128, c, 0:tw],
                                     start=(c == 0), stop=(c == EJ - 1))
                q = qpool.tile([128, TC], BF16, tag=f"q{j}", name=f"q{j}")
                nc.vector.tensor_scalar_add(q[0:128, 0:tw], qp[0:128, 0:tw],
                                            bqt[:, j:j + 1])
                qs.append(q)
            return qs

        # ---- packed k/q staging for straddling heads: their two score
        # fragments collapse to ONE matmul against a q tile whose rows
        # 0:80 are the head's features (repacked via SBUF-SBUF DMA).
        kpk = {}
        qpk = {}
        for (h, fiA, fiB, j, p0, p1) in STRADDLE:
            kpk[h] = cpool.tile([128, S], BF16, tag=f"kpk{h}", name=f"kpk{h}")
            qpk[h] = cpool.tile([128, TC], BF16, tag=f"qpk{h}",
                                name=f"qpk{h}")
            nc.gpsimd.memset(kpk[h][:], 0.0)
            nc.gpsimd.memset(qpk[h][:], 0.0)

        def emit_kpk():
            for (h, fiA, fiB, j, p0, p1) in STRADDLE:
                nc.sync.dma_start(kpk[h][0:128 - p0, :],
                                  kstage[fiA][p0:128, :])
                nc.sync.dma_start(kpk[h][128 - p0:D, :],
                                  kstage[fiB][0:p1, :])

        def emit_qpk(qs, tw):
            for (h, fiA, fiB, j, p0, p1) in STRADDLE:
                nc.sync.dma_start(qpk[h][0:128 - p0, 0:tw],
                                  qs[j][p0:128, 0:tw])
                nc.sync.dma_start(qpk[h][128 - p0:D, 0:tw],
                                  qs[j + 1][0:p1, 0:tw])

        def _softmax_tail(h, sc, tw, aps):
            """exp -> allreduce(sumexp) -> approx-recip -> normalize.
            The normalize multiplies alternate DVE/Pool to spread the
            elementwise load (DVE is near-saturated in steady state)."""
            a = apool.tile([S, TC], BF16, tag=f"a{h}", name=f"a{h}")
            nc.scalar.activation(a[0:S, 0:tw], sc[0:S, 0:tw], AF.Exp,
                                 scale=SCALE)
            ar = arpool.tile([S, TC], F32, tag="ar")
            nc.gpsimd.partition_all_reduce(
                ar[0:S, 0:tw], a[0:S, 0:tw], channels=S,
                reduce_op=bass_isa.ReduceOp.add)
            nc.vector.reciprocal_approx_fast(ar[0:S, 0:tw], ar[0:S, 0:tw])
            eng = nc.vector if h % 2 == 0 else nc.gpsimd
            eng.tensor_mul(a[0:S, 0:tw], a[0:S, 0:tw], ar[0:S, 0:tw])
            aps[h] = a

        def emit_scores_all(qs, tw, aps):
            """fragment-accumulation scores for all heads (chunk 0 only,
            before the packed q staging pipeline is primed)."""
            for h in range(H):
                frs = [(fi, f) for fi, f in enumerate(FRAGS) if f[0] == h]
                sc = psc.tile([S, TC], F32, tag="sc")
                for i, (fi, (hh, j, p0, p1)) in enumerate(frs):
                    nc.tensor.matmul(sc[0:S, 0:tw], kstage[fi][:],
                                     qs[j][0:128, 0:tw],
                                     start=(i == 0), stop=(i == len(frs) - 1))
                _softmax_tail(h, sc, tw, aps)

        def emit_scores_singles(qs, tw, aps):
            for (h, fi) in SINGLE:
                j = FRAGS[fi][1]
                sc = psc.tile([S, TC], F32, tag="sc")
                nc.tensor.matmul(sc[0:S, 0:tw], kstage[fi][:],
                                 qs[j][0:128, 0:tw], start=True, stop=True)
                _softmax_tail(h, sc, tw, aps)

        def emit_scores_packed(tw, aps):
            for (h, fiA, fiB, j, p0, p1) in STRADDLE:
                sc = psc.tile([S, TC], F32, tag="sc")
                nc.tensor.matmul(sc[0:S, 0:tw], kpk[h][:],
                                 qpk[h][0:128, 0:tw], start=True, stop=True)
                _softmax_tail(h, sc, tw, aps)

        def emit_av_tile(j, aps, tw, dve_at=False):
            av = pav.tile([128, TC], F32, tag="av")
            frs = [(fi, f) for fi, f in enumerate(FRAGS) if f[1] == j]
            for i, (fi, (h, jj, p0, p1)) in enumerate(frs):
                nc.tensor.matmul(av[0:128, 0:tw],
                                 vb[:, 128 * fi:128 * (fi + 1)],
                                 aps[h][0:S, 0:tw],
                                 start=(i == 0), stop=(i == len(frs) - 1))
            at = atpool.tile([128, TC], BF16, tag=f"at{j}", name=f"at{j}")
            if j < AT_ACT_N and not dve_at:
                nc.scalar.activation(at[0:128, 0:tw], av[0:128, 0:tw],
                                     AF.Copy)
            else:
                nc.vector.tensor_copy(at[0:128, 0:tw], av[0:128, 0:tw])
            return at

        def emit_oproj(attn, t0, tw, store_eng=None):
            obt = opool.tile([128, EJ, TC], BF16, tag="ob")
            for p in range(EJ):
                op = pop.tile([128, TC], F32, tag="op")
                for j in range(EJ):
                    nc.tensor.matmul(op[0:128, 0:tw],
                                     wo_t[:, j, 128 * p:128 * (p + 1)],
                                     attn[j][0:128, 0:tw],
                                     start=(j == 0), stop=(j == EJ - 1))
                nc.scalar.activation(obt[0:128, p, 0:tw], op[0:128, 0:tw],
                                     AF.Identity, bias=bot[:, p:p + 1])
            # store on the ACT HWDGE ring: frees the SP ring for x/qpk
            # prefetches and pairs naturally with the ob copies above
            (store_eng or nc.scalar).dma_start(otr[:, :, t0:t0 + tw],
                                               obt[0:128, 0:EJ, 0:tw])

        # ---- setup PE work: Vproj (needs yt+wv), Kproj (wk), Q(0) ----
        emit_vproj()
        emit_kproj()
        emit_kpk()
        qs = emit_qproj(xp0, CHUNKS[0][1])
        wo_t = loadw("wo", dr["wo"], EJ, E, eng=nc.scalar)

        attn_prev = None
        tprev = None
        xps = [xp0, xp1]
        for it in range(NT):
            t0, tw = CHUNKS[it]
            # prefetch x(i+2) (x0/x1 loaded during setup)
            if it + 2 < NT:
                nt0, ntw = CHUNKS[it + 2]
                xp_next = xpool.tile([128, EJ, TC], BF16, tag="xp")
                nc.sync.dma_start(xp_next[0:128, 0:EJ, 0:ntw],
                                  xtr[:, :, nt0:nt0 + ntw])
                xps.append(xp_next)

            aps = {}
            if it == 0:
                emit_scores_all(qs, tw, aps)
            else:
                emit_scores_singles(qs, tw, aps)
                emit_scores_packed(tw, aps)

            if it > 0:
                # near the narrow tail chunks the ACT SEQ is congested;
                # route those stores back to the SP ring
                emit_oproj(attn_prev, tprev[0], tprev[1],
                           store_eng=nc.sync if it >= NT - 2 else None)

            if it < NT - 1:
                # Q(i+1) before AV(i): together with Oproj(i-1) it covers
                # the softmax normalization chain latency of chunk i.
                qs = emit_qproj(xps[it + 1], CHUNKS[it + 1][1])
                emit_qpk(qs, CHUNKS[it + 1][1])
                attn = [emit_av_tile(j, aps, tw) for j in range(EJ)]
            else:
                # last chunk: j-outer Oproj interleaved with AV tiles,
                # staggered by one so the at-copy latency is hidden.
                ops = [pop.tile([128, TC], F32, tag="op", name=f"opl{p}")
                       for p in range(2)]
                ops += [pq.tile([128, TC], F32, tag="qp", name=f"opl{p + 2}")
                        for p in range(2)]
                ops += [psc.tile([128, TC], F32, tag="sc", name="opl4")]
                attn = []
                obt = opool.tile([128, EJ, TC], BF16, tag="ob")

                def last_o_group(j):
                    for p in range(EJ):
                        nc.tensor.matmul(ops[p][0:128, 0:tw],
                                         wo_t[:, j, 128 * p:128 * (p + 1)],
                                         attn[j][0:128, 0:tw],
                                         start=(j == 0), stop=(j == EJ - 1))

                for j in range(EJ):
                    # at-copies on DVE: it is idle in the last chunk (no
                    # Q(i+1) bias work) while ACT still drains exp's
                    attn.append(emit_av_tile(j, aps, tw, dve_at=True))
                    if j >= 1:
                        last_o_group(j - 1)
                # final accumulation group: emit each p's closing matmul,
                # its psum->sbuf bias copy, and the store as soon as ready
                for p in range(EJ):
                    nc.tensor.matmul(ops[p][0:128, 0:tw],
                                     wo_t[:, EJ - 1, 128 * p:128 * (p + 1)],
                                     attn[EJ - 1][0:128, 0:tw],
                                     start=False, stop=True)
                    if p % 2 == 0:
                        nc.scalar.activation(obt[0:128, p, 0:tw],
                                             ops[p][0:128, 0:tw],
                                             AF.Identity,
                                             bias=bot[:, p:p + 1])
                    else:
                        nc.vector.tensor_scalar_add(obt[0:128, p, 0:tw],
                                                    ops[p][0:128, 0:tw],
                                                    bot[:, p:p + 1])
                    if p == 2:
                        nc.scalar.dma_start(otr[:, 0:3, t0:t0 + tw],
                                            obt[0:128, 0:3, 0:tw])
                nc.sync.dma_start(otr[:, 3:EJ, t0:t0 + tw],
                                  obt[0:128, 3:EJ, 0:tw])
            attn_prev, tprev = attn, (t0, tw)


def build_program():
    nc = bacc.Bacc("TRN2", target_bir_lowering=False, debug=False,
                   num_devices=B)
    dr = {}

    def din(name, shape, dt):
        dr[name] = nc.dram_tensor(name, shape, dt, kind="ExternalInput")
        return dr[name]

    din("xt", [E, T], BF16)
    din("yt", [CR, S], BF16)
    din("wq", [E, E], BF16)
    din("wk", [CR, E], BF16)
    din("wv", [CR, E], BF16)
    din("wo", [E, E], BF16)
    din("consts", [128, 27], F32)
    din("consts2", [1, E + S], BF16)
    dr["ot"] = nc.dram_tensor("ot", [E, T], BF16, kind="ExternalOutput")

    with tile.TileContext(nc) as tc:
        _emit(nc, tc, {k: v[:] for k, v in dr.items()})
    nc.compile()
    return nc


def make_in_maps(x, y, Wq, bq, Wk, bk, Wv, bv, Wo, bo):
    import ml_dtypes
    BF = ml_dtypes.bfloat16

    def fb(a):
        return np.ascontiguousarray(np.asarray(a, np.float32).astype(BF))

    consts = np.zeros((128, 27), np.float32)
    consts[:, 0:EJ] = np.asarray(bq, np.float32).reshape(EJ, 128).T
    consts[:, EJ:2 * EJ] = np.asarray(bk, np.float32).reshape(EJ, 128).T
    consts[:, 2 * EJ:3 * EJ] = np.asarray(bo, np.float32).reshape(EJ, 128).T
    for fi, (h, j, p0, p1) in enumerate(FRAGS):
        consts[p0:p1, 3 * EJ + fi] = 1.0
    consts2 = np.zeros((1, E + S), np.float32)
    consts2[0, 0:E] = np.asarray(bv, np.float32)
    consts2[0, E:E + S] = 1.0

    shared = dict(
        wq=fb(Wq), wk=fb(Wk), wv=fb(Wv), wo=fb(Wo),
        consts=consts, consts2=fb(consts2),
    )
    x = np.asarray(x, np.float32)
    y = np.asarray(y, np.float32)
    in_maps = []
    for b in range(B):
        m = dict(shared)
        m["xt"] = fb(x[b].T)
        m["yt"] = fb(y[b].T)
        in_maps.append(m)
    return in_maps


def assemble_output(results):
    return np.stack(
        [np.asarray(results[b]["ot"]).astype(np.float32).T
         for b in range(B)], axis=0)


_PROG = None


def _prog():
    global _PROG
    if _PROG is None:
        _PROG = build_program()
    return _PROG


def kernel(x, y, Wq, bq, Wk, bk, Wv, bv, Wo, bo):
    nc = _prog()
    in_maps = make_in_maps(x, y, Wq, bq, Wk, bk, Wv, bv, Wo, bo)
    res = run_bass_kernel_spmd(nc, in_maps, core_ids=list(range(B)))
    return assemble_output(res.results)
